# revision 40
# baseline (speedup 1.0000x reference)
"""Trainium2 Bass kernel for one burst-mode CIF neuron step.

Reference math (closed form of the two burst while-loops):
    m      = mem + x
    q      = m / th
    k_pos  = max(ceil(q) - 1, 0)
    j_mem  = max(-floor(q) - 1, 0)          (mutually exclusive with k_pos)
    k_neg  = min(j_mem, round(spike_count/th))
    spike  = (k_pos - k_neg) * th

Device reformulation.  Let g = ceil(q) = rint(q + 0.5) a.e. and
s = spike_count/th >= 0.  Then
    k_pos   = relu(g - 1)
    -k_neg  = max(min(g, 0), -s)
    spike   = th * (k_pos - k_neg)

The rint rides the f16 OUTPUT ROUNDING of one ACT op: for |v| < 512,
f16(v + 1536) = 1536 + rint(v) (f16 ulp is 1.0 on [1024, 2048)).  So
    ta_b = f16(R*m + 1536.5)        = 1536 + g          (one ACT op)
    kp_b = max(ta_b, 1537)          = 1537 + k_pos      (DVE TS, 4x)
    jn_b = min(ta_b, 1536)          = 1536 + min(g,0)   (DVE TS, 4x)
    sn_b = f16(sc*(-R) + 1536)      = 1536 - s          (DVE TS, 4x)
    kn_b = max(jn_b, sn_b)          = 1536 - k_neg      (DVE TT, 2x)
    psum = I.T@kp_b + I.T@kn_b      = 3073 + d          (PE, exact ints)
    out  = bf16(th*psum - 3073*th)  = th * d            (ACT, per-part bias)
All intermediates are exact small integers (+bias) in f16; the only
rounding error sources are the f16 input quantization and the bf16
output (measured end-to-end rel err 1.1e-2 vs the f32 reference,
gate 2e-2).

Layout: TRANSPOSED so the hidden dim lives on partitions.  Rows
(B*T = 16384) are sharded 8-way data-parallel (2048 rows/core = free
dim); H = 4096 becomes 32 partition-blocks of 128.  Threshold is then a
per-partition [128,1] scalar per block, so every (1/th) multiply fuses
into tensor_scalar / ACT scale-bias operands.  Input arrives packed
[x | mem | sc] per partition-row: one contiguous 1.5MB DMA per block.

GPSIMD is deliberately IDLE: its tensor_scalar ucode runs ~21 cyc/elem
(~36us per block) and, while active, starves the DVE via the shared
SBUF port (measured: identical DVE ops 1.45us -> 35us when GPSIMD
runs).  Total per-core HBM traffic 64MB (~188us roofline at 358GB/s).
"""

import numpy as np

B, T, H = 4, 4096, 4096
N_CORES = 8
R_TOTAL = B * T            # 16384 rows
FD = R_TOTAL // N_CORES    # 2048 rows per core = free dim
P = 128
NB = H // P                # 32 h-blocks per core
C16 = 1536.0               # f16 rint magic: 1.5 * 2^10
NMM = 512                  # matmul free-dim per PSUM bank

_NC_CACHE: dict = {}


def build_nc():
    """Build the per-core Bass program (identical on all cores)."""
    from contextlib import ExitStack

    import concourse.bacc as bacc
    import concourse.bass as bass
    import concourse.mybir as mybir
    from concourse.tile import TileContext

    f32 = mybir.dt.float32
    f16 = mybir.dt.float16
    bf16 = mybir.dt.bfloat16
    f8 = mybir.dt.float8e4
    Alu = mybir.AluOpType
    Act = mybir.ActivationFunctionType

    nc = bacc.Bacc("TRN2", target_bir_lowering=False, debug=False)
    # per partition-row, per block PAIR:
    #   x0_f16 | x1_f16 | mem0_f16 | mem1_f16 | sc0_fp8 | sc1_fp8
    # so the pair-fused TT ops see [2 @ stride FD, FD @ 1] patterns.
    xm_d = nc.dram_tensor("xm", [(NB // 2) * P, 5 * FD], bf16, kind="ExternalInput").ap()
    th_d = nc.dram_tensor("thp", [P, NB], f32, kind="ExternalInput").ap()
    e_d = nc.dram_tensor("eye", [P, P], f16, kind="ExternalInput").ap()
    # per pair row: [spike0 | spike1]
    o_d = nc.dram_tensor("spike", [(NB // 2) * P, 2 * FD], bf16, kind="ExternalOutput").ap()

    with TileContext(nc) as tc, ExitStack() as ctx:
        consts = ctx.enter_context(tc.tile_pool(name="consts", bufs=1))
        io = ctx.enter_context(tc.tile_pool(name="io", bufs=4))
        pm = ctx.enter_context(tc.tile_pool(name="pm", bufs=2))
        pa = ctx.enter_context(tc.tile_pool(name="pa", bufs=2))
        pk = ctx.enter_context(tc.tile_pool(name="pk", bufs=2))
        pj = ctx.enter_context(tc.tile_pool(name="pj", bufs=2))
        ps = ctx.enter_context(tc.tile_pool(name="ps", bufs=2))
        po = ctx.enter_context(tc.tile_pool(name="po", bufs=3))
        psum = ctx.enter_context(tc.tile_pool(name="psum", bufs=4, space="PSUM"))

        # ---- one-time setup ----
        TH = consts.tile([P, NB], f32, tag="TH")
        nc.sync.dma_start(out=TH[:], in_=th_d)
        Rr = consts.tile([P, NB], f32, tag="Rr")
        nc.vector.reciprocal(Rr[:], TH[:])
        Rn = consts.tile([P, NB], f32, tag="Rn")
        nc.vector.tensor_scalar_mul(Rn[:], Rr[:], -1.0)
        BTH = consts.tile([P, NB], f32, tag="BTH")
        nc.vector.tensor_scalar_mul(BTH[:], TH[:], -(2.0 * C16 + 1.0))
        eye = consts.tile([P, P], f16, tag="eye")
        nc.sync.dma_start(out=eye[:], in_=e_d)
        bias_sn = consts.tile([P, 1], f32, tag="bias_sn")
        nc.vector.memset(bias_sn[:], C16)

        xm_t = xm_d.rearrange("(ng p) w -> ng p w", p=P)  # ng = block PAIR
        o_pair = o_d.rearrange("(ng p) w -> ng p w", p=P)

        # One 2.5MB HWDGE DMA per block PAIR.  The two TT ops (m, kn)
        # fuse across the pair (no per-partition scalars involved); the
        # scalar-carrying ops (ta, sn, out) stay per-block.
        for g in range(NB // 2):
            txm = io.tile([P, 5 * FD], bf16, tag="xm")
            nc.sync.dma_start(out=txm[:], in_=xm_t[g])
            xpair = txm[:, 0 : 2 * FD].bitcast(f16)
            mpair = txm[:, 2 * FD : 4 * FD].bitcast(f16)

            # m = x + mem  (DVE TT f16 2x, both blocks in one pass)
            tm = pm.tile([P, 2 * FD], f16, tag="m")
            nc.vector.tensor_tensor(tm[:], xpair, mpair, Alu.add)

            ta = pa.tile([P, 2 * FD], f16, tag="ta")
            sn = ps.tile([P, 2 * FD], f16, tag="sn")
            for hf in range(2):
                b = 2 * g + hf
                sl = bass.ts(hf, FD)
                # ta_b = f16(R*m + 1536.5) = 1536 + ceil(q)  (DVE TS 4x;
                # the f16 output rounding IS the rint)
                nc.vector.tensor_scalar(
                    ta[:, sl], tm[:, sl], Rr[:, b : b + 1], C16 + 0.5,
                    Alu.mult, Alu.add,
                )
                # sn_b = sc*(-R) + 1536 = 1536 - s  (split: ACT takes the
                # first SPL columns, DVE the rest, balancing the two walls)
                scp = txm[
                    :, 4 * FD + hf * (FD // 2) : 4 * FD + (hf + 1) * (FD // 2)
                ].bitcast(f8)
                SPL = 1792
                nc.scalar.activation(
                    sn[:, hf * FD : hf * FD + SPL], scp[:, 0:SPL], Act.Identity,
                    bias=bias_sn[:], scale=Rn[:, b : b + 1],
                )
                nc.vector.tensor_scalar(
                    sn[:, hf * FD + SPL : (hf + 1) * FD], scp[:, SPL:FD],
                    Rn[:, b : b + 1], C16, Alu.mult, Alu.add,
                )
            # kp_b = max(ta_b, 1537) = 1537 + k_pos  (DVE TS 4x, pair-wide)
            kp = pk.tile([P, 2 * FD], f16, tag="kp")
            nc.vector.tensor_scalar_max(kp[:], ta[:], C16 + 1.0)
            # jn_b = min(ta_b, 1536) = 1536 + min(g,0)  (DVE TS 4x, pair-wide)
            jn = pj.tile([P, 2 * FD], f16, tag="jn")
            nc.vector.tensor_scalar_min(jn[:], ta[:], C16)
            # kn_b = max(jn_b, sn_b) = 1536 - k_neg  (DVE TT, pair-wide)
            nc.vector.tensor_tensor(jn[:], jn[:], sn[:], Alu.max)

            tout = po.tile([P, 2 * FD], bf16, tag="out")
            for hf in range(2):
                b = 2 * g + hf
                # psum = I.T@kp_b + I.T@kn_b = 3073 + d  (PE; exact ints)
                # Two 2-bank PSUM tiles per block for a finer PE->ACT handoff.
                for h in range(2):
                    td = psum.tile([P, FD // 2], f32, tag="td")
                    for c in range(FD // 2 // NMM):
                        cs = bass.ts(hf * 4 + h * 2 + c, NMM)
                        ds = bass.ts(c, NMM)
                        nc.tensor.matmul(
                            td[:, ds], eye[:], kp[:, cs], start=True, stop=False
                        )
                        nc.tensor.matmul(
                            td[:, ds], eye[:], jn[:, cs], start=False, stop=True
                        )
                    # spike = th*psum - 3073*th = th*d  (ACT: PSUM->SBUF)
                    nc.scalar.activation(
                        tout[:, bass.ts(hf * 2 + h, FD // 2)],
                        td[:],
                        Act.Identity,
                        bias=BTH[:, b : b + 1],
                        scale=TH[:, b : b + 1],
                    )
            # one 1MB out-DMA per pair via SWDGE: keeps the output stream
            # off the input HWDGE ring; GPSIMD is otherwise idle so
            # descriptor-gen there is free.
            nc.gpsimd.dma_start(out=o_pair[g], in_=tout[:])

    return nc


def make_in_maps(x, mem, sc, th):
    """Pack full [R_TOTAL, H] inputs into per-core transposed tensors.

    Per core: xm[b, p, :] = [x_f16 | mem_f16 | sc_bf16] for hidden channel
    h = b*128+p over that core's 2048 rows, so each block is one
    contiguous DMA.
    """
    import ml_dtypes

    x16 = x.astype(np.float16)
    m16 = mem.astype(np.float16)
    s8 = sc.astype(ml_dtypes.float8_e4m3)
    thp = np.ascontiguousarray(th.reshape(NB, P).T)  # [P, NB] f32
    eye = np.eye(P, dtype=np.float16)

    in_maps = []
    for c in range(N_CORES):
        rs = slice(c * FD, (c + 1) * FD)
        # packed per pair row: [x0 | x1 | mem0 | mem1 | sc0 | sc1]
        xt = x16[rs].view(np.uint16).reshape(FD, NB, P)  # [FD, NB, P]
        mt = m16[rs].view(np.uint16).reshape(FD, NB, P)
        pkd = np.empty((NB // 2, P, 5 * FD), dtype=np.uint16)
        pkd[:, :, 0:FD] = xt[:, 0::2].transpose(1, 2, 0)
        pkd[:, :, FD : 2 * FD] = xt[:, 1::2].transpose(1, 2, 0)
        pkd[:, :, 2 * FD : 3 * FD] = mt[:, 0::2].transpose(1, 2, 0)
        pkd[:, :, 3 * FD : 4 * FD] = mt[:, 1::2].transpose(1, 2, 0)
        sct = np.ascontiguousarray(
            s8[rs].view(np.uint8).reshape(FD, NB, P).transpose(1, 2, 0)
        ).view(np.uint16)  # [NB, P, FD/2] u16
        pkd[:, :, 4 * FD : 9 * FD // 2] = sct[0::2]
        pkd[:, :, 9 * FD // 2 : 5 * FD] = sct[1::2]
        in_maps.append(
            {
                "xm": pkd.reshape((NB // 2) * P, 5 * FD).view(ml_dtypes.bfloat16),
                "thp": thp,
                "eye": eye,
            }
        )
    return in_maps


def unpack_out(results):
    """results[c]["spike"] [NB*P, FD] bf16 -> full [B, T, H] f32."""
    outs = []
    for c in range(N_CORES):
        sp = np.asarray(results[c]["spike"]).astype(np.float32)
        # [NB/2, P, 2, FD] -> [NB, P, FD] -> [FD, NB, P] -> [FD, H]
        sp = sp.reshape(NB // 2, P, 2, FD).swapaxes(1, 2).reshape(NB, P, FD)
        outs.append(sp.transpose(2, 0, 1).reshape(FD, H))
    return np.concatenate(outs, axis=0).reshape(B, T, H)


def kernel(**inputs: np.ndarray) -> np.ndarray:
    from concourse.bass_utils import run_bass_kernel_spmd

    x = np.ascontiguousarray(inputs["x"], dtype=np.float32).reshape(R_TOTAL, H)
    mem = np.ascontiguousarray(inputs["mem"], dtype=np.float32).reshape(R_TOTAL, H)
    sc = np.ascontiguousarray(inputs["spike_count"], dtype=np.float32).reshape(
        R_TOTAL, H
    )
    th = np.ascontiguousarray(inputs["threshold"], dtype=np.float32)

    if "nc" not in _NC_CACHE:
        nc = build_nc()
        nc.finalize()
        _NC_CACHE["nc"] = nc
    nc = _NC_CACHE["nc"]

    in_maps = make_in_maps(x, mem, sc, th)
    res = run_bass_kernel_spmd(nc, in_maps, core_ids=list(range(N_CORES)))
    return unpack_out(res.results)


# revision 42
# speedup vs baseline: 1.2220x; 1.2220x over previous
"""Trainium2 Bass kernel for one burst-mode CIF neuron step.

Reference math (closed form of the two burst while-loops):
    m      = mem + x
    q      = m / th
    k_pos  = max(ceil(q) - 1, 0)
    j_mem  = max(-floor(q) - 1, 0)          (mutually exclusive with k_pos)
    k_neg  = min(j_mem, round(spike_count/th))
    spike  = (k_pos - k_neg) * th

Device reformulation.  Let g = ceil(q) = rint(q + 0.5) a.e. and
s = spike_count/th >= 0.  Then
    k_pos   = relu(g - 1)
    -k_neg  = max(min(g, 0), -s)
    spike   = th * (k_pos - k_neg)

The rint rides the f16 OUTPUT ROUNDING of one ACT op: for |v| < 512,
f16(v + 1536) = 1536 + rint(v) (f16 ulp is 1.0 on [1024, 2048)).  So
    ta_b = f16(R*m + 1536.5)        = 1536 + g          (one ACT op)
    kp_b = max(ta_b, 1537)          = 1537 + k_pos      (DVE TS, 4x)
    jn_b = min(ta_b, 1536)          = 1536 + min(g,0)   (DVE TS, 4x)
    sn_b = f16(sc*(-R) + 1536)      = 1536 - s          (DVE TS, 4x)
    kn_b = max(jn_b, sn_b)          = 1536 - k_neg      (DVE TT, 2x)
    psum = I.T@kp_b + I.T@kn_b      = 3073 + d          (PE, exact ints)
    out  = bf16(th*psum - 3073*th)  = th * d            (ACT, per-part bias)
All intermediates are exact small integers (+bias) in f16; the only
rounding error sources are the f16 input quantization and the bf16
output (measured end-to-end rel err 1.1e-2 vs the f32 reference,
gate 2e-2).

Layout: TRANSPOSED so the hidden dim lives on partitions.  Rows
(B*T = 16384) are sharded 8-way data-parallel (2048 rows/core = free
dim); H = 4096 becomes 32 partition-blocks of 128.  Threshold is then a
per-partition [128,1] scalar per block, so every (1/th) multiply fuses
into tensor_scalar / ACT scale-bias operands.  Input arrives packed
[x0|x1|mem0|mem1|sc0|sc1] per block PAIR: one contiguous 2.5MB HWDGE
DMA; outputs leave as 1MB SWDGE DMAs (GPSIMD descriptor-gen) so the
output stream never queues behind inputs on the HWDGE FIFO ring.

Engine split (measured busy/core at 165-190us e2e): DVE 137us (m, ta,
kp, jn, kn; the TT ops pair-fused), ACT 145us (sn from fp8, 2x out
stage), PE 126us (256 matmuls), DMA ~143us active at ~350GB/s = the
HBM-per-core roofline for the 58.8MB of traffic.

GPSIMD runs NO compute: its tensor_scalar ucode is ~21 cyc/elem (~36us
per block) and, while active, starves the DVE via the shared SBUF port
(measured: identical DVE ops 1.45us -> 35us when GPSIMD streams).
"""

import numpy as np

B, T, H = 4, 4096, 4096
N_CORES = 8
R_TOTAL = B * T            # 16384 rows
FD = R_TOTAL // N_CORES    # 2048 rows per core = free dim
P = 128
NB = H // P                # 32 h-blocks per core
C16 = 1536.0               # f16 rint magic: 1.5 * 2^10
NMM = 512                  # matmul free-dim per PSUM bank

_NC_CACHE: dict = {}


def build_nc():
    """Build the per-core Bass program (identical on all cores)."""
    from contextlib import ExitStack

    import concourse.bacc as bacc
    import concourse.bass as bass
    import concourse.mybir as mybir
    from concourse.tile import TileContext

    f32 = mybir.dt.float32
    f16 = mybir.dt.float16
    bf16 = mybir.dt.bfloat16
    f8 = mybir.dt.float8e4
    Alu = mybir.AluOpType
    Act = mybir.ActivationFunctionType

    nc = bacc.Bacc("TRN2", target_bir_lowering=False, debug=False)
    # per partition-row, per block PAIR:
    #   x0_f16 | x1_f16 | mem0_f16 | mem1_f16 | sc0_fp8 | sc1_fp8
    # so the pair-fused TT ops see [2 @ stride FD, FD @ 1] patterns.
    xm_d = nc.dram_tensor("xm", [(NB // 2) * P, 5 * FD], bf16, kind="ExternalInput").ap()
    th_d = nc.dram_tensor("thp", [P, NB], f32, kind="ExternalInput").ap()
    e_d = nc.dram_tensor("eye", [P, P], f16, kind="ExternalInput").ap()
    # per pair row: [spike0 | spike1]
    o_d = nc.dram_tensor("spike", [(NB // 2) * P, 2 * FD], bf16, kind="ExternalOutput").ap()

    with TileContext(nc) as tc, ExitStack() as ctx:
        consts = ctx.enter_context(tc.tile_pool(name="consts", bufs=1))
        io = ctx.enter_context(tc.tile_pool(name="io", bufs=4))
        pm = ctx.enter_context(tc.tile_pool(name="pm", bufs=2))
        pa = ctx.enter_context(tc.tile_pool(name="pa", bufs=2))
        pk = ctx.enter_context(tc.tile_pool(name="pk", bufs=2))
        pj = ctx.enter_context(tc.tile_pool(name="pj", bufs=2))
        ps = ctx.enter_context(tc.tile_pool(name="ps", bufs=2))
        po = ctx.enter_context(tc.tile_pool(name="po", bufs=3))
        psum = ctx.enter_context(tc.tile_pool(name="psum", bufs=4, space="PSUM"))

        # ---- one-time setup ----
        TH = consts.tile([P, NB], f32, tag="TH")
        nc.sync.dma_start(out=TH[:], in_=th_d)
        Rr = consts.tile([P, NB], f32, tag="Rr")
        nc.vector.reciprocal(Rr[:], TH[:])
        Rn = consts.tile([P, NB], f32, tag="Rn")
        nc.vector.tensor_scalar_mul(Rn[:], Rr[:], -1.0)
        BTH = consts.tile([P, NB], f32, tag="BTH")
        nc.vector.tensor_scalar_mul(BTH[:], TH[:], -(2.0 * C16 + 1.0))
        eye = consts.tile([P, P], f16, tag="eye")
        nc.sync.dma_start(out=eye[:], in_=e_d)
        bias_sn = consts.tile([P, 1], f32, tag="bias_sn")
        nc.vector.memset(bias_sn[:], C16)

        xm_t = xm_d.rearrange("(ng p) w -> ng p w", p=P)  # ng = block PAIR
        o_pair = o_d.rearrange("(ng p) w -> ng p w", p=P)

        # One 2.5MB HWDGE DMA per block PAIR.  The two TT ops (m, kn)
        # fuse across the pair (no per-partition scalars involved); the
        # scalar-carrying ops (ta, sn, out) stay per-block.
        for g in range(NB // 2):
            txm = io.tile([P, 5 * FD], bf16, tag="xm")
            nc.sync.dma_start(out=txm[:], in_=xm_t[g])
            xpair = txm[:, 0 : 2 * FD].bitcast(f16)
            mpair = txm[:, 2 * FD : 4 * FD].bitcast(f16)

            # m = x + mem  (DVE TT f16 2x, both blocks in one pass)
            tm = pm.tile([P, 2 * FD], f16, tag="m")
            nc.vector.tensor_tensor(tm[:], xpair, mpair, Alu.add)

            ta = pa.tile([P, 2 * FD], f16, tag="ta")
            sn = ps.tile([P, 2 * FD], f16, tag="sn")
            for hf in range(2):
                b = 2 * g + hf
                sl = bass.ts(hf, FD)
                # ta_b = f16(R*m + 1536.5) = 1536 + ceil(q)  (DVE TS 4x;
                # the f16 output rounding IS the rint)
                nc.vector.tensor_scalar(
                    ta[:, sl], tm[:, sl], Rr[:, b : b + 1], C16 + 0.5,
                    Alu.mult, Alu.add,
                )
                # sn_b = sc*(-R) + 1536 = 1536 - s  (ACT; fp8 src)
                scp = txm[
                    :, 4 * FD + hf * (FD // 2) : 4 * FD + (hf + 1) * (FD // 2)
                ].bitcast(f8)
                nc.scalar.activation(
                    sn[:, sl], scp, Act.Identity,
                    bias=bias_sn[:], scale=Rn[:, b : b + 1],
                )
            # kp_b = max(ta_b, 1537) = 1537 + k_pos  (DVE TS 4x, pair-wide)
            kp = pk.tile([P, 2 * FD], f16, tag="kp")
            nc.vector.tensor_scalar_max(kp[:], ta[:], C16 + 1.0)
            # jn_b = min(ta_b, 1536) = 1536 + min(g,0)  (DVE TS 4x, pair-wide)
            jn = pj.tile([P, 2 * FD], f16, tag="jn")
            nc.vector.tensor_scalar_min(jn[:], ta[:], C16)
            # kn_b = max(jn_b, sn_b) = 1536 - k_neg  (DVE TT, pair-wide)
            nc.vector.tensor_tensor(jn[:], jn[:], sn[:], Alu.max)

            tout = po.tile([P, 2 * FD], bf16, tag="out")
            for hf in range(2):
                b = 2 * g + hf
                # psum = I.T@kp_b + I.T@kn_b = 3073 + d  (PE; exact ints)
                # Two 2-bank PSUM tiles per block for a finer PE->ACT handoff.
                for h in range(2):
                    td = psum.tile([P, FD // 2], f32, tag="td")
                    for c in range(FD // 2 // NMM):
                        cs = bass.ts(hf * 4 + h * 2 + c, NMM)
                        ds = bass.ts(c, NMM)
                        nc.tensor.matmul(
                            td[:, ds], eye[:], kp[:, cs], start=True, stop=False
                        )
                        nc.tensor.matmul(
                            td[:, ds], eye[:], jn[:, cs], start=False, stop=True
                        )
                    # spike = th*psum - 3073*th = th*d  (ACT: PSUM->SBUF)
                    nc.scalar.activation(
                        tout[:, bass.ts(hf * 2 + h, FD // 2)],
                        td[:],
                        Act.Identity,
                        bias=BTH[:, b : b + 1],
                        scale=TH[:, b : b + 1],
                    )
            # one 1MB out-DMA per pair via SWDGE: keeps the output stream
            # off the input HWDGE ring; GPSIMD is otherwise idle so
            # descriptor-gen there is free.
            nc.gpsimd.dma_start(out=o_pair[g], in_=tout[:])

    return nc


def make_in_maps(x, mem, sc, th):
    """Pack full [R_TOTAL, H] inputs into per-core transposed tensors.

    Per core: xm[b, p, :] = [x_f16 | mem_f16 | sc_bf16] for hidden channel
    h = b*128+p over that core's 2048 rows, so each block is one
    contiguous DMA.
    """
    import ml_dtypes

    x16 = x.astype(np.float16)
    m16 = mem.astype(np.float16)
    s8 = sc.astype(ml_dtypes.float8_e4m3)
    thp = np.ascontiguousarray(th.reshape(NB, P).T)  # [P, NB] f32
    eye = np.eye(P, dtype=np.float16)

    in_maps = []
    for c in range(N_CORES):
        rs = slice(c * FD, (c + 1) * FD)
        # packed per pair row: [x0 | x1 | mem0 | mem1 | sc0 | sc1]
        xt = x16[rs].view(np.uint16).reshape(FD, NB, P)  # [FD, NB, P]
        mt = m16[rs].view(np.uint16).reshape(FD, NB, P)
        pkd = np.empty((NB // 2, P, 5 * FD), dtype=np.uint16)
        pkd[:, :, 0:FD] = xt[:, 0::2].transpose(1, 2, 0)
        pkd[:, :, FD : 2 * FD] = xt[:, 1::2].transpose(1, 2, 0)
        pkd[:, :, 2 * FD : 3 * FD] = mt[:, 0::2].transpose(1, 2, 0)
        pkd[:, :, 3 * FD : 4 * FD] = mt[:, 1::2].transpose(1, 2, 0)
        sct = np.ascontiguousarray(
            s8[rs].view(np.uint8).reshape(FD, NB, P).transpose(1, 2, 0)
        ).view(np.uint16)  # [NB, P, FD/2] u16
        pkd[:, :, 4 * FD : 9 * FD // 2] = sct[0::2]
        pkd[:, :, 9 * FD // 2 : 5 * FD] = sct[1::2]
        in_maps.append(
            {
                "xm": pkd.reshape((NB // 2) * P, 5 * FD).view(ml_dtypes.bfloat16),
                "thp": thp,
                "eye": eye,
            }
        )
    return in_maps


def unpack_out(results):
    """results[c]["spike"] [NB*P, FD] bf16 -> full [B, T, H] f32."""
    outs = []
    for c in range(N_CORES):
        sp = np.asarray(results[c]["spike"]).astype(np.float32)
        # [NB/2, P, 2, FD] -> [NB, P, FD] -> [FD, NB, P] -> [FD, H]
        sp = sp.reshape(NB // 2, P, 2, FD).swapaxes(1, 2).reshape(NB, P, FD)
        outs.append(sp.transpose(2, 0, 1).reshape(FD, H))
    return np.concatenate(outs, axis=0).reshape(B, T, H)


def kernel(**inputs: np.ndarray) -> np.ndarray:
    from concourse.bass_utils import run_bass_kernel_spmd

    x = np.ascontiguousarray(inputs["x"], dtype=np.float32).reshape(R_TOTAL, H)
    mem = np.ascontiguousarray(inputs["mem"], dtype=np.float32).reshape(R_TOTAL, H)
    sc = np.ascontiguousarray(inputs["spike_count"], dtype=np.float32).reshape(
        R_TOTAL, H
    )
    th = np.ascontiguousarray(inputs["threshold"], dtype=np.float32)

    if "nc" not in _NC_CACHE:
        nc = build_nc()
        nc.finalize()
        _NC_CACHE["nc"] = nc
    nc = _NC_CACHE["nc"]

    in_maps = make_in_maps(x, mem, sc, th)
    res = run_bass_kernel_spmd(nc, in_maps, core_ids=list(range(N_CORES)))
    return unpack_out(res.results)


# revision 45
# speedup vs baseline: 1.2396x; 1.0144x over previous
"""Trainium2 Bass kernel for one burst-mode CIF neuron step.

Reference math (closed form of the two burst while-loops):
    m      = mem + x
    q      = m / th
    k_pos  = max(ceil(q) - 1, 0)
    j_mem  = max(-floor(q) - 1, 0)          (mutually exclusive with k_pos)
    k_neg  = min(j_mem, round(spike_count/th))
    spike  = (k_pos - k_neg) * th

Device reformulation.  Let g = ceil(q) = rint(q + 0.5) a.e. and
s = spike_count/th >= 0.  Then
    k_pos   = relu(g - 1)
    -k_neg  = max(min(g, 0), -s)
    spike   = th * (k_pos - k_neg)

The rint rides the f16 OUTPUT ROUNDING of one ACT op: for |v| < 512,
f16(v + 1536) = 1536 + rint(v) (f16 ulp is 1.0 on [1024, 2048)).  So
    ta_b = f16(R*m + 1536.5)        = 1536 + g          (one ACT op)
    kp_b = max(ta_b, 1537)          = 1537 + k_pos      (DVE TS, 4x)
    jn_b = min(ta_b, 1536)          = 1536 + min(g,0)   (DVE TS, 4x)
    sn_b = f16(sc*(-R) + 1536)      = 1536 - s          (DVE TS, 4x)
    kn_b = max(jn_b, sn_b)          = 1536 - k_neg      (DVE TT, 2x)
    psum = I.T@kp_b + I.T@kn_b      = 3073 + d          (PE, exact ints)
    out  = bf16(th*psum - 3073*th)  = th * d            (ACT, per-part bias)
All intermediates are exact small integers (+bias) in f16; the only
rounding error sources are the f16 input quantization and the bf16
output (measured end-to-end rel err 1.1e-2 vs the f32 reference,
gate 2e-2).

Layout: TRANSPOSED so the hidden dim lives on partitions.  Rows
(B*T = 16384) are sharded 8-way data-parallel (2048 rows/core = free
dim); H = 4096 becomes 32 partition-blocks of 128.  Threshold is then a
per-partition [128,1] scalar per block, so every (1/th) multiply fuses
into tensor_scalar / ACT scale-bias operands.  Input arrives packed
[x0|x1|mem0|mem1|sc0|sc1] per block PAIR: one contiguous 2.5MB HWDGE
DMA; outputs leave as 1MB SWDGE DMAs (GPSIMD descriptor-gen) so the
output stream never queues behind inputs on the HWDGE FIFO ring.

Engine split (measured busy/core at 165-190us e2e): DVE 137us (m, ta,
kp, jn, kn; the TT ops pair-fused), ACT 145us (sn from fp8, 2x out
stage), PE 126us (256 matmuls), DMA ~143us active at ~350GB/s = the
HBM-per-core roofline for the 58.8MB of traffic.

GPSIMD runs NO compute: its tensor_scalar ucode is ~21 cyc/elem (~36us
per block) and, while active, starves the DVE via the shared SBUF port
(measured: identical DVE ops 1.45us -> 35us when GPSIMD streams).
"""

import numpy as np

B, T, H = 4, 4096, 4096
N_CORES = 8
R_TOTAL = B * T            # 16384 rows
FD = R_TOTAL // N_CORES    # 2048 rows per core = free dim
P = 128
NB = H // P                # 32 h-blocks per core
C16 = 1536.0               # f16 rint magic: 1.5 * 2^10
NMM = 512                  # matmul free-dim per PSUM bank

_NC_CACHE: dict = {}


def build_nc():
    """Build the per-core Bass program (identical on all cores)."""
    from contextlib import ExitStack

    import concourse.bacc as bacc
    import concourse.bass as bass
    import concourse.mybir as mybir
    from concourse.tile import TileContext

    f32 = mybir.dt.float32
    f16 = mybir.dt.float16
    bf16 = mybir.dt.bfloat16
    f8 = mybir.dt.float8e4
    Alu = mybir.AluOpType
    Act = mybir.ActivationFunctionType

    nc = bacc.Bacc("TRN2", target_bir_lowering=False, debug=False)
    # per partition-row, per block PAIR:
    #   x0_f16 | x1_f16 | mem0_f16 | mem1_f16 | sc0_fp8 | sc1_fp8
    # so the pair-fused TT ops see [2 @ stride FD, FD @ 1] patterns.
    xm_d = nc.dram_tensor("xm", [(NB // 2) * P, 5 * FD], bf16, kind="ExternalInput").ap()
    th_d = nc.dram_tensor("thp", [P, NB], f32, kind="ExternalInput").ap()
    e_d = nc.dram_tensor("eye", [P, P], f16, kind="ExternalInput").ap()
    # per pair row: [spike0 | spike1]
    o_d = nc.dram_tensor("spike", [(NB // 2) * P, 2 * FD], bf16, kind="ExternalOutput").ap()

    with TileContext(nc) as tc, ExitStack() as ctx:
        consts = ctx.enter_context(tc.tile_pool(name="consts", bufs=1))
        io = ctx.enter_context(tc.tile_pool(name="io", bufs=4))
        pm = ctx.enter_context(tc.tile_pool(name="pm", bufs=2))
        pa = ctx.enter_context(tc.tile_pool(name="pa", bufs=2))
        pk = ctx.enter_context(tc.tile_pool(name="pk", bufs=2))
        pj = ctx.enter_context(tc.tile_pool(name="pj", bufs=2))
        ps = ctx.enter_context(tc.tile_pool(name="ps", bufs=2))
        po = ctx.enter_context(tc.tile_pool(name="po", bufs=4))
        psum = ctx.enter_context(tc.tile_pool(name="psum", bufs=4, space="PSUM"))

        # ---- one-time setup ----
        TH = consts.tile([P, NB], f32, tag="TH")
        nc.sync.dma_start(out=TH[:], in_=th_d)
        Rr = consts.tile([P, NB], f32, tag="Rr")
        nc.vector.reciprocal(Rr[:], TH[:])
        Rn = consts.tile([P, NB], f32, tag="Rn")
        nc.vector.tensor_scalar_mul(Rn[:], Rr[:], -1.0)
        BTH = consts.tile([P, NB], f32, tag="BTH")
        nc.vector.tensor_scalar_mul(BTH[:], TH[:], -(2.0 * C16 + 1.0))
        eye = consts.tile([P, P], f16, tag="eye")
        nc.sync.dma_start(out=eye[:], in_=e_d)
        bias_sn = consts.tile([P, 1], f32, tag="bias_sn")
        nc.vector.memset(bias_sn[:], C16)

        xm_t = xm_d.rearrange("(ng p) w -> ng p w", p=P)  # ng = block PAIR
        o_pair = o_d.rearrange("(ng p) w -> ng p w", p=P)

        # One 2.5MB HWDGE DMA per block PAIR.  The two TT ops (m, kn)
        # fuse across the pair (no per-partition scalars involved); the
        # scalar-carrying ops (ta, sn, out) stay per-block.
        for g in range(NB // 2):
            txm = io.tile([P, 5 * FD], bf16, tag="xm")
            if g == 0:
                # split the first fill so the m-TT starts ~1.5us earlier
                nc.sync.dma_start(out=txm[:, 0 : 4 * FD], in_=xm_t[g][:, 0 : 4 * FD])
                nc.sync.dma_start(out=txm[:, 4 * FD : 5 * FD], in_=xm_t[g][:, 4 * FD : 5 * FD])
            else:
                nc.sync.dma_start(out=txm[:], in_=xm_t[g])
            xpair = txm[:, 0 : 2 * FD].bitcast(f16)
            mpair = txm[:, 2 * FD : 4 * FD].bitcast(f16)

            # m = x + mem  (DVE TT f16 2x, both blocks in one pass)
            tm = pm.tile([P, 2 * FD], f16, tag="m")
            nc.vector.tensor_tensor(tm[:], xpair, mpair, Alu.add)

            ta = pa.tile([P, 2 * FD], f16, tag="ta")
            sn = ps.tile([P, 2 * FD], f16, tag="sn")
            for hf in range(2):
                b = 2 * g + hf
                sl = bass.ts(hf, FD)
                # ta_b = f16(R*m + 1536.5) = 1536 + ceil(q)  (DVE TS 4x;
                # the f16 output rounding IS the rint)
                nc.vector.tensor_scalar(
                    ta[:, sl], tm[:, sl], Rr[:, b : b + 1], C16 + 0.5,
                    Alu.mult, Alu.add,
                )
                # sn_b = sc*(-R) + 1536 = 1536 - s  (ACT; fp8 src)
                scp = txm[
                    :, 4 * FD + hf * (FD // 2) : 4 * FD + (hf + 1) * (FD // 2)
                ].bitcast(f8)
                nc.scalar.activation(
                    sn[:, sl], scp, Act.Identity,
                    bias=bias_sn[:], scale=Rn[:, b : b + 1],
                )
            # kp_b = max(ta_b, 1537) = 1537 + k_pos  (DVE TS 4x, pair-wide)
            kp = pk.tile([P, 2 * FD], f16, tag="kp")
            nc.vector.tensor_scalar_max(kp[:], ta[:], C16 + 1.0)
            # jn_b = min(ta_b, 1536) = 1536 + min(g,0)  (DVE TS 4x, pair-wide)
            jn = pj.tile([P, 2 * FD], f16, tag="jn")
            nc.vector.tensor_scalar_min(jn[:], ta[:], C16)
            # kn_b = max(jn_b, sn_b) = 1536 - k_neg  (DVE TT, pair-wide)
            nc.vector.tensor_tensor(jn[:], jn[:], sn[:], Alu.max)

            tout = po.tile([P, 2 * FD], bf16, tag="out")
            for hf in range(2):
                b = 2 * g + hf
                # psum = I.T@kp_b + I.T@kn_b = 3073 + d  (PE; exact ints)
                # Two 2-bank PSUM tiles per block for a finer PE->ACT handoff.
                for h in range(2):
                    td = psum.tile([P, FD // 2], f32, tag="td")
                    for c in range(FD // 2 // NMM):
                        cs = bass.ts(hf * 4 + h * 2 + c, NMM)
                        ds = bass.ts(c, NMM)
                        nc.tensor.matmul(
                            td[:, ds], eye[:], kp[:, cs], start=True, stop=False
                        )
                        nc.tensor.matmul(
                            td[:, ds], eye[:], jn[:, cs], start=False, stop=True
                        )
                    # spike = th*psum - 3073*th = th*d  (ACT: PSUM->SBUF)
                    nc.scalar.activation(
                        tout[:, bass.ts(hf * 2 + h, FD // 2)],
                        td[:],
                        Act.Identity,
                        bias=BTH[:, b : b + 1],
                        scale=TH[:, b : b + 1],
                    )
            # out-DMAs via SWDGE: keeps the output stream off the input
            # HWDGE ring; GPSIMD is otherwise idle so descriptor-gen there
            # is free.  1MB per pair mid-stream; the last two pairs go out
            # per block (0.5MB) so the tail drains sooner.
            if g >= NB // 2 - 2:
                nc.gpsimd.dma_start(out=o_pair[g][:, 0:FD], in_=tout[:, 0:FD])
                nc.gpsimd.dma_start(
                    out=o_pair[g][:, FD : 2 * FD], in_=tout[:, FD : 2 * FD]
                )
            else:
                nc.gpsimd.dma_start(out=o_pair[g], in_=tout[:])

    return nc


def make_in_maps(x, mem, sc, th):
    """Pack full [R_TOTAL, H] inputs into per-core transposed tensors.

    Per core: xm[b, p, :] = [x_f16 | mem_f16 | sc_bf16] for hidden channel
    h = b*128+p over that core's 2048 rows, so each block is one
    contiguous DMA.
    """
    import ml_dtypes

    x16 = x.astype(np.float16)
    m16 = mem.astype(np.float16)
    s8 = sc.astype(ml_dtypes.float8_e4m3)
    thp = np.ascontiguousarray(th.reshape(NB, P).T)  # [P, NB] f32
    eye = np.eye(P, dtype=np.float16)

    in_maps = []
    for c in range(N_CORES):
        rs = slice(c * FD, (c + 1) * FD)
        # packed per pair row: [x0 | x1 | mem0 | mem1 | sc0 | sc1]
        xt = x16[rs].view(np.uint16).reshape(FD, NB, P)  # [FD, NB, P]
        mt = m16[rs].view(np.uint16).reshape(FD, NB, P)
        pkd = np.empty((NB // 2, P, 5 * FD), dtype=np.uint16)
        pkd[:, :, 0:FD] = xt[:, 0::2].transpose(1, 2, 0)
        pkd[:, :, FD : 2 * FD] = xt[:, 1::2].transpose(1, 2, 0)
        pkd[:, :, 2 * FD : 3 * FD] = mt[:, 0::2].transpose(1, 2, 0)
        pkd[:, :, 3 * FD : 4 * FD] = mt[:, 1::2].transpose(1, 2, 0)
        sct = np.ascontiguousarray(
            s8[rs].view(np.uint8).reshape(FD, NB, P).transpose(1, 2, 0)
        ).view(np.uint16)  # [NB, P, FD/2] u16
        pkd[:, :, 4 * FD : 9 * FD // 2] = sct[0::2]
        pkd[:, :, 9 * FD // 2 : 5 * FD] = sct[1::2]
        in_maps.append(
            {
                "xm": pkd.reshape((NB // 2) * P, 5 * FD).view(ml_dtypes.bfloat16),
                "thp": thp,
                "eye": eye,
            }
        )
    return in_maps


def unpack_out(results):
    """results[c]["spike"] [NB*P, FD] bf16 -> full [B, T, H] f32."""
    outs = []
    for c in range(N_CORES):
        sp = np.asarray(results[c]["spike"]).astype(np.float32)
        # [NB/2, P, 2, FD] -> [NB, P, FD] -> [FD, NB, P] -> [FD, H]
        sp = sp.reshape(NB // 2, P, 2, FD).swapaxes(1, 2).reshape(NB, P, FD)
        outs.append(sp.transpose(2, 0, 1).reshape(FD, H))
    return np.concatenate(outs, axis=0).reshape(B, T, H)


def kernel(**inputs: np.ndarray) -> np.ndarray:
    from concourse.bass_utils import run_bass_kernel_spmd

    x = np.ascontiguousarray(inputs["x"], dtype=np.float32).reshape(R_TOTAL, H)
    mem = np.ascontiguousarray(inputs["mem"], dtype=np.float32).reshape(R_TOTAL, H)
    sc = np.ascontiguousarray(inputs["spike_count"], dtype=np.float32).reshape(
        R_TOTAL, H
    )
    th = np.ascontiguousarray(inputs["threshold"], dtype=np.float32)

    if "nc" not in _NC_CACHE:
        nc = build_nc()
        nc.finalize()
        _NC_CACHE["nc"] = nc
    nc = _NC_CACHE["nc"]

    in_maps = make_in_maps(x, mem, sc, th)
    res = run_bass_kernel_spmd(nc, in_maps, core_ids=list(range(N_CORES)))
    return unpack_out(res.results)
